# revision 9
# baseline (speedup 1.0000x reference)
"""TRN2 Bass kernel for nn_BSquareModelCombined (spiking MLP, LIF neurons).

Strategy (v3)
-------------
The reference scans over T=100 steps, but the GEMMs are state-independent:
  h1 = x_t @ W1^T  for all t  -> one big GEMM over R = T*B_loc rows
  LIF scan (elementwise) -> spikes s1
  h2 = s1 @ W2^T   -> one big GEMM;  LIF scan -> s2
  h3 = s2 @ W3^T   -> small GEMM; output-layer scan + voting on host.

Data-parallel over batch: 8 cores x 4 batch rows, feature-major on-chip
layout [D, R] with r = t*BL + b.

Measured HW facts driving the design:
 - PE period = max(moving 400 rows @ 0.4167ns, LDWEIGHTS + ~40ns).
   LDW: bf16/f32r-class fast (~97-140ns), fp16/fp8 half-rate (~162ns),
   so f32r stationary (12-bit, ~185ns/call) beats fp16 (~204ns/call).
 - fp8 DoubleRow contracts 2 k-tiles per ~203ns call: best $/k-tile for
   low-order corrections (spikes are +-1, exact in e4m3).
 - Walrus rejects mixing 32-bit (f32r) with 16/8-bit matmul operands;
   e4m3 moving x fp16 stationary is supported and exact.

Precision (h1 needs fp32-class, h2 ~1e-5, h3 ~1e-3):
 - GEMM1: exactly the f32r 12-bit hi/lo trick: (xhi+xlo)@Whi + xhi@Wlo,
   error ~1.6e-7 rms.
 - GEMM2: spikes g=+-1 e4m3; W2/2 = round12 hi (f32r... but f32r needs
   f32r moving, so the hi pass ships W2 hi as f32r and spikes ALSO as
   f32r?  No: hi pass uses f32r stationary with f32r moving spikes is
   too much SBUF; instead hi = round12 shipped f32r with moving spikes
   kept e4m3 is illegal -> hi pass uses f32r stationary + f32r moving
   is rejected; we use g8 e4m3 moving x W2hi FP16 stationary when
   G2_HI='fp16', or gr f32r moving x W2hi f32r when G2_HI='f32r'.
   Default 'f32r': spikes also materialized as f32r for the hi pass.
   Lo pass: e4m3((W2/2 - hi) * 2^23) via fp8 DoubleRow, combined as
   h2 = psHi + psLo * 2^-23 + bias''  (bias from fp64 host rowsums).
 - GEMM3: single fp16 pass (mixed with e4m3 spikes; h3 budget loose).

LIF scan: 5 feature groups [10,10,10,10,5]; fused DVE op per step
  m_t = beta*m + h_t - (m > 0)  into an 8-slot ring; Sign on the
Scalar engine emits 4 timesteps per op (amortizes scalar overhead).
"""
import sys

sys.path.insert(0, "/opt/trn_rl_repo")
sys.path.insert(0, "/root/.axon_site")

import numpy as np
import ml_dtypes

import concourse.bass as bass  # noqa: F401
import concourse.tile as tile
from concourse import bacc, mybir
from concourse import dve_ops
from concourse.dve_spec import Spec, Src0, Src1, C0, Zero, lower as dve_lower
from concourse.dve_uop import DveOpSpec
from concourse.bass_utils import run_bass_kernel_spmd

F32 = mybir.dt.float32
F32R = mybir.dt.float32r
FP16 = mybir.dt.float16
E4M3 = mybir.dt.float8e4
DR = mybir.MatmulPerfMode.DoubleRow
Ident = mybir.ActivationFunctionType.Identity

B, T_FULL, DIN, DH, DOUT = 32, 100, 2312, 5760, 90
NCORES = 8
BL = B // NCORES            # batch rows per core
KP = 19                     # D_in tiles after padding 2312 -> 2432
DINP = KP * 128
MT = DH // 128              # 45 feature tiles
KT2 = MT - 1                # k-tiles covered by DR pairs (44) + 1 single
GRP = [12, 12, 12, 4, 4, 1]  # scan chunk groups (fine tail -> short lag)
G0 = np.cumsum([0] + GRP)
BETA, THRESH = 0.9, 1.0
NUM_CLASSES, TRI_NUM = 10, 45
S_W2L = 2.0**23             # W2 residual prescale

G2_HI = "fp16"              # 'fp16' (e4m3 moving) — f32r rejected w/ e4m3

_nc_cache = {}
_prep_cache = {}


def _register_lif_op():
    """Fused LIF membrane update: out = s0*in0 + in1 - (in0 > 0)."""
    name = "LIF_STEP_ANT"
    for o in dve_ops.OPS:
        if o.name == name:
            return o
    spec = Spec(
        body=(Src0 * C0) + Src1 - (Src0 > Zero),
        reference=lambda in0, in1, s0, s1, imm2: in0.astype(np.float32) * s0
        + in1.reshape(in0.shape)
        - (in0 > 0).astype(np.float32),
    )
    row = max(dve_ops._SUB_OPCODE_FOR_NAME.values()) + 1
    shas = {}
    for ver in ("v3", "v4"):
        uops = dve_lower(spec, ver=ver)
        shas[ver] = DveOpSpec(name=name, opcode=row, uops=uops, rd1_en=True).sha(ver)
    op = dve_ops.DveOp(name, spec, subdim=False, uops_sha=shas)
    dve_ops.OPS.append(op)
    dve_ops.CUSTOM_DVE_SPECS[name] = spec
    dve_ops._SUB_OPCODE_FOR_NAME[name] = row
    return op


LIF_OP = _register_lif_op()


def _build(T):
    """Build + compile the per-core program (same program on all 8 cores)."""
    R = T * BL
    nc = bacc.Bacc(None, target_bir_lowering=False)

    XCH = [(0, 2), (2, 7), (7, 12), (12, 19)]
    xhi_ds = [nc.dram_tensor(f"xhi{i}", [128, c1 - c0, R], F32R,
                             kind="ExternalInput")
              for i, (c0, c1) in enumerate(XCH)]
    xlo_ds = [nc.dram_tensor(f"xlo{i}", [128, c1 - c0, R], F32R,
                             kind="ExternalInput")
              for i, (c0, c1) in enumerate(XCH)]
    whi_d = nc.dram_tensor("whi", [MT, 128, KP, 128], F32R, kind="ExternalInput")
    wlo_d = nc.dram_tensor("wlo", [MT, 128, KP, 128], F32R, kind="ExternalInput")
    w2h_d = nc.dram_tensor("w2h", [MT, 128, MT, 128], FP16, kind="ExternalInput")
    w2l_d = nc.dram_tensor("w2l", [MT, 128, MT, 128], E4M3,
                           kind="ExternalInput")
    w3_d = nc.dram_tensor("w3", [128, MT, DOUT], FP16, kind="ExternalInput")
    b12_d = nc.dram_tensor("b12", [128, 2 * MT], F32, kind="ExternalInput")
    h3o = nc.dram_tensor("h3", [DOUT, R], F32, kind="ExternalOutput")

    add, mult = mybir.AluOpType.add, mybir.AluOpType.mult

    with tile.TileContext(nc) as tc:
        with (
            tc.tile_pool(name="acts", bufs=1) as acts,
            tc.tile_pool(name="wpool", bufs=4) as wpool,
            tc.tile_pool(name="small", bufs=1) as small,
            tc.tile_pool(name="psum", bufs=2, space="PSUM") as pp,
        ):
            xhi = acts.tile([128, KP, R], F32R, tag="xhi")
            xlo = acts.tile([128, KP, R], F32R, tag="xlo")
            hsb = acts.tile([128, MT, R], F32, tag="h")
            b12sb = small.tile([128, 2 * MT], F32)
            sts = [small.tile([128, GRP[g], 8 * BL], F32, name=f"st{g}")
                   for g in range(len(GRP))]

            nc.sync.dma_start(out=b12sb[:], in_=b12_d.ap())
            # x pieces: contiguous per-chunk tensors on the gpsimd queue
            for i, (c0, c1) in enumerate(XCH):
                nc.gpsimd.dma_start(out=xhi[:, c0:c1, :], in_=xhi_ds[i].ap())
            for i, (c0, c1) in enumerate(XCH):
                nc.gpsimd.dma_start(out=xlo[:, c0:c1, :], in_=xlo_ds[i].ap())

            # ---- GEMM1 pass 1+2: (xhi + xlo) @ Whi ----
            with nc.named_scope("gemm1a"):
                for m in range(MT):
                    whi = wpool.tile([128, KP, 128], F32R, tag="w")
                    nc.sync.dma_start(out=whi[:], in_=whi_d.ap()[m])
                    ps = pp.tile([128, R], F32, tag="psA")
                    for k in range(KP):
                        nc.tensor.matmul(
                            ps[:], whi[:, k, :], xhi[:, k, :],
                            start=(k == 0), stop=False,
                        )
                    for k in range(KP):
                        nc.tensor.matmul(
                            ps[:], whi[:, k, :], xlo[:, k, :],
                            start=False, stop=(k == KP - 1),
                        )
                    # hsb = ps + bias1 on the Scalar engine (keeps DVE free)
                    nc.scalar.activation(
                        hsb[:, m, :], ps[:], Ident,
                        bias=b12sb[:, m : m + 1], scale=1.0,
                    )

            # spikes, sign-encoded (+1/-1), e4m3 (exact in fp8)
            g1sb = acts.tile([128, MT, R], E4M3, tag="xlo")  # alias: xlo dead
            g2sb = acts.tile([128, MT, R], E4M3, tag="xhi")  # alias: xhi dead

            # ---- GEMM1 pass 3: xhi @ Wlo, accumulated into hsb ----
            with nc.named_scope("gemm1b"):
                for m in range(MT):
                    wlo = wpool.tile([128, KP, 128], F32R, tag="w")
                    nc.sync.dma_start(out=wlo[:], in_=wlo_d.ap()[m])
                    ps = pp.tile([128, R], F32, tag="psB")
                    for k in range(KP):
                        nc.tensor.matmul(
                            ps[:], wlo[:, k, :], xhi[:, k, :],
                            start=(k == 0), stop=(k == KP - 1),
                        )
                    nc.vector.tensor_tensor(
                        hsb[:, m, :], hsb[:, m, :], ps[:], op=add
                    )

            def lif_scan(scope, gsb):
                # group g depends only on its own feature tiles, so it
                # starts as soon as the producing GEMM has evicted tiles
                # [G0[g], G0[g+1]) and hides under the GEMM tail.
                with nc.named_scope(scope):
                    for g, gw in enumerate(GRP):
                        st = sts[g]
                        nc.vector.memset(st[:, :, 7 * BL : 8 * BL], -1.0)
                        for t in range(T):
                            hsl = hsb[:, G0[g] : G0[g + 1],
                                      BL * t : BL * (t + 1)]
                            so, do = ((t - 1) % 8) * BL, (t % 8) * BL
                            nc.vector._custom_dve(
                                LIF_OP,
                                out=st[:, :, do : do + BL],
                                in0=st[:, :, so : so + BL],
                                in1=hsl, s0=BETA,
                            )
                            if t % 4 == 3:
                                bo = ((t - 3) % 8) * BL
                                nc.scalar.sign(
                                    gsb[:, G0[g] : G0[g + 1],
                                        BL * (t - 3) : BL * (t + 1)],
                                    st[:, :, bo : bo + 4 * BL],
                                )

            lif_scan("scan1", g1sb)

            # ---- GEMM2: chainA = g1 @ W2hi (fp16); chainB = DR fp8 lo ----
            with nc.named_scope("gemm2"):
                for m in range(MT):
                    w2ht = wpool.tile([128, MT, 128], FP16, tag="w")
                    w2lt = wpool.tile([128, MT, 128], E4M3, tag="w")
                    nc.sync.dma_start(out=w2ht[:], in_=w2h_d.ap()[m])
                    nc.sync.dma_start(out=w2lt[:], in_=w2l_d.ap()[m])
                    psA = pp.tile([128, R], F32, tag="psA")
                    for k in range(MT):
                        nc.tensor.matmul(
                            psA[:], w2ht[:, k, :], g1sb[:, k, :],
                            start=(k == 0), stop=(k == MT - 1),
                        )
                    psB = pp.tile([128, R], F32, tag="psB")
                    for j in range(KT2 // 2):
                        nc.tensor.matmul(
                            psB[:], w2lt[:, 2 * j : 2 * j + 2, :],
                            g1sb[:, 2 * j : 2 * j + 2, :],
                            start=(j == 0), stop=False,
                            perf_mode=DR,
                        )
                    nc.tensor.matmul(
                        psB[:], w2lt[:, MT - 1, :], g1sb[:, MT - 1, :],
                        start=False, stop=True,
                    )
                    nc.vector.tensor_scalar(
                        hsb[:, m, :], psB[:], 1.0 / S_W2L,
                        b12sb[:, MT + m : MT + m + 1], mult, add,
                    )
                    nc.vector.tensor_tensor(
                        hsb[:, m, :], hsb[:, m, :], psA[:], op=add
                    )

            lif_scan("scan2", g2sb)

            # ---- GEMM3: h3 = g2 @ W3h (mixed, single pass), out [90, R] ----
            with nc.named_scope("gemm3"):
                w3sb = wpool.tile([128, MT, DOUT], FP16, tag="w")
                nc.sync.dma_start(out=w3sb[:], in_=w3_d.ap())
                ps3 = pp.tile([DOUT, R], F32, tag="ps3")
                for k in range(MT):
                    nc.tensor.matmul(
                        ps3[:], w3sb[:, k, :], g2sb[:, k, :],
                        start=(k == 0), stop=(k == MT - 1),
                    )
                h3sb = small.tile([DOUT, R], F32, tag="h3sb")
                nc.vector.tensor_copy(h3sb[:], ps3[:])
                nc.sync.dma_start(out=h3o.ap(), in_=h3sb[:])

    nc.compile()
    return nc


def _f16(a):
    return np.asarray(a, np.float32).astype(np.float16)


def _e4m3(a):
    return np.asarray(a, np.float32).astype(ml_dtypes.float8_e4m3)


def _round12(a):
    """Round fp32 to 12-bit significand, RNE — the f32r PE operand grid."""
    u = np.ascontiguousarray(a, np.float32).view(np.uint32)
    u = (u + 0x7FF + ((u >> 12) & 1)) & np.uint32(0xFFFFF000)
    return u.view(np.float32)


def _prep_weights(fc1_w, fc1_b, fc2_w, fc2_b, fco_w):
    key = (fc1_w.ctypes.data, fc2_w.ctypes.data, fco_w.ctypes.data)
    if key in _prep_cache:
        return _prep_cache[key]
    # GEMM1: 12-bit hi/lo split of W1^T tiles (f32r)
    W1p = np.zeros((DH, DINP), np.float32)
    W1p[:, :DIN] = fc1_w
    W1t = np.ascontiguousarray(
        W1p.reshape(MT, 128, KP, 128).transpose(0, 3, 2, 1)
    )  # [m, p, k, q] = W1[m*128+q, k*128+p]
    whi = _round12(W1t)
    wlo = np.ascontiguousarray(W1t - whi)  # exact 12-bit residual
    # GEMM2: W2/2 = fp16 hi + e4m3(residual * 2^23), sign-encoded spikes
    W2t = (fc2_w.reshape(MT, 128, MT, 128).transpose(0, 3, 2, 1)
           .astype(np.float64) * 0.5)
    w2h = np.ascontiguousarray(_f16(W2t))
    w2l45 = _e4m3((W2t - w2h.astype(np.float64)) * S_W2L)  # [m, p, 45, q]
    w2l = np.ascontiguousarray(w2l45)
    # GEMM3: W3/2 single fp16
    W3t = (fco_w.reshape(DOUT, MT, 128).transpose(2, 1, 0)
           .astype(np.float64) * 0.5)  # [p, k, q]
    w3 = np.ascontiguousarray(_f16(W3t))
    # biases: threshold shift -(1-beta); sign-encoding rowsum corrections
    # from the actually-shipped weight values (fp64 for exactness)
    c2 = (w2h.astype(np.float64)
          + w2l45.astype(np.float64) / S_W2L).sum(axis=(1, 2))  # [m, q]
    b2c = (fc2_b.astype(np.float64).reshape(MT, 128)
           - (1.0 - BETA) * THRESH + c2).astype(np.float32)
    b1s = (fc1_b - (1.0 - BETA) * THRESH).reshape(MT, 128).T
    b12 = np.ascontiguousarray(
        np.concatenate([b1s, b2c.T], axis=1).astype(np.float32))
    b3c = w3.astype(np.float64).sum(axis=(0, 1))  # [DOUT]
    out = dict(
        inputs=dict(whi=whi, wlo=wlo, w2h=w2h, w2l=w2l, w3=w3, b12=b12),
        b3c=b3c.astype(np.float32),
    )
    _prep_cache[key] = out
    return out


def _prep_x(x, T):
    """Per-core x arrays [128, KP, R] (r = t*BL + b), 12-bit hi/lo split."""
    xf = np.asarray(x, np.float32).reshape(B, T, -1)
    outs = []
    for c in range(NCORES):
        xc = xf[BL * c : BL * (c + 1)]            # [BL, T, DIN]
        xp = np.zeros((DINP, T * BL), np.float32)
        xp[:DIN] = xc.transpose(2, 1, 0).reshape(DIN, T * BL)
        xt = np.ascontiguousarray(
            xp.reshape(KP, 128, T * BL).transpose(1, 0, 2))
        xhi = _round12(xt)
        xlo = np.ascontiguousarray(xt - xhi)
        XCH = [(0, 2), (2, 7), (7, 12), (12, 19)]
        d = {}
        for i, (c0, c1) in enumerate(XCH):
            d[f"xhi{i}"] = np.ascontiguousarray(xhi[:, c0:c1, :])
            d[f"xlo{i}"] = np.ascontiguousarray(xlo[:, c0:c1, :])
        outs.append(d)
    return outs


def kernel(x, fc1_w, fc1_b, fc2_w, fc2_b, fco_w, fco_b, _T=None,
           _want_results=False, _trace=False):
    T = _T or T_FULL
    if T not in _nc_cache:
        _nc_cache[T] = _build(T)
    nc = _nc_cache[T]

    w = _prep_weights(
        np.asarray(fc1_w, np.float32), np.asarray(fc1_b, np.float32),
        np.asarray(fc2_w, np.float32), np.asarray(fc2_b, np.float32),
        np.asarray(fco_w, np.float32),
    )
    xs = _prep_x(x, T)
    in_maps = [{**xs[c], **w["inputs"]} for c in range(NCORES)]
    res = run_bass_kernel_spmd(nc, in_maps, list(range(NCORES)), trace=_trace)

    # host: output-layer LIF scan + T-sum + pairwise voting (exact fp32)
    h3 = np.stack([res.results[c]["h3"] for c in range(NCORES)])  # [8, 90, R]
    i3 = h3.reshape(NCORES, DOUT, T, BL) \
        + (np.asarray(fco_b, np.float32) + w["b3c"])[None, :, None, None]
    i3 = i3.transpose(2, 0, 3, 1).reshape(T, B, DOUT)  # [T, 32, 90]
    m = np.zeros((B, DOUT), np.float32)
    s = np.zeros((B, DOUT), np.float32)
    out = np.zeros((B, DOUT), np.float32)
    for t in range(T):
        m = BETA * m + i3[t] - s * THRESH
        s = ((m - THRESH) > 0).astype(np.float32)
        out += s
    pi, pj = np.triu_indices(NUM_CLASSES, 1)
    outp = out.reshape(B, TRI_NUM, 2)
    votes = np.zeros((B, NUM_CLASSES), np.float32)
    np.add.at(votes, (slice(None), pi), outp[..., 0])
    np.add.at(votes, (slice(None), pj), outp[..., 1])
    if _want_results:
        return votes, res
    return votes


# revision 10
# speedup vs baseline: 1.0703x; 1.0703x over previous
"""TRN2 Bass kernel for nn_BSquareModelCombined (spiking MLP, LIF neurons).

Strategy (v3)
-------------
The reference scans over T=100 steps, but the GEMMs are state-independent:
  h1 = x_t @ W1^T  for all t  -> one big GEMM over R = T*B_loc rows
  LIF scan (elementwise) -> spikes s1
  h2 = s1 @ W2^T   -> one big GEMM;  LIF scan -> s2
  h3 = s2 @ W3^T   -> small GEMM; output-layer scan + voting on host.

Data-parallel over batch: 8 cores x 4 batch rows, feature-major on-chip
layout [D, R] with r = t*BL + b.

Measured HW facts driving the design:
 - PE period = max(moving 400 rows @ 0.4167ns, LDWEIGHTS + ~40ns).
   LDW: bf16/f32r-class fast (~97-140ns), fp16/fp8 half-rate (~162ns),
   so f32r stationary (12-bit, ~185ns/call) beats fp16 (~204ns/call).
 - fp8 DoubleRow contracts 2 k-tiles per ~203ns call: best $/k-tile for
   low-order corrections (spikes are +-1, exact in e4m3).
 - Walrus rejects mixing 32-bit (f32r) with 16/8-bit matmul operands;
   e4m3 moving x fp16 stationary is supported and exact.

Precision (h1 needs fp32-class, h2 ~1e-5, h3 ~1e-3):
 - GEMM1: exactly the f32r 12-bit hi/lo trick: (xhi+xlo)@Whi + xhi@Wlo,
   error ~1.6e-7 rms.
 - GEMM2: spikes g=+-1 e4m3; W2/2 = round12 hi (f32r... but f32r needs
   f32r moving, so the hi pass ships W2 hi as f32r and spikes ALSO as
   f32r?  No: hi pass uses f32r stationary with f32r moving spikes is
   too much SBUF; instead hi = round12 shipped f32r with moving spikes
   kept e4m3 is illegal -> hi pass uses f32r stationary + f32r moving
   is rejected; we use g8 e4m3 moving x W2hi FP16 stationary when
   G2_HI='fp16', or gr f32r moving x W2hi f32r when G2_HI='f32r'.
   Default 'f32r': spikes also materialized as f32r for the hi pass.
   Lo pass: e4m3((W2/2 - hi) * 2^23) via fp8 DoubleRow, combined as
   h2 = psHi + psLo * 2^-23 + bias''  (bias from fp64 host rowsums).
 - GEMM3: single fp16 pass (mixed with e4m3 spikes; h3 budget loose).

LIF scan: 5 feature groups [10,10,10,10,5]; fused DVE op per step
  m_t = beta*m + h_t - (m > 0)  into an 8-slot ring; Sign on the
Scalar engine emits 4 timesteps per op (amortizes scalar overhead).
"""
import sys

sys.path.insert(0, "/opt/trn_rl_repo")
sys.path.insert(0, "/root/.axon_site")

import numpy as np
import ml_dtypes

import concourse.bass as bass  # noqa: F401
import concourse.tile as tile
from concourse import bacc, mybir
from concourse import dve_ops
from concourse.dve_spec import Spec, Src0, Src1, C0, Zero, lower as dve_lower
from concourse.dve_uop import DveOpSpec
from concourse.bass_utils import run_bass_kernel_spmd

F32 = mybir.dt.float32
F32R = mybir.dt.float32r
FP16 = mybir.dt.float16
E4M3 = mybir.dt.float8e4
DR = mybir.MatmulPerfMode.DoubleRow
Ident = mybir.ActivationFunctionType.Identity

B, T_FULL, DIN, DH, DOUT = 32, 100, 2312, 5760, 90
NCORES = 8
BL = B // NCORES            # batch rows per core
KP = 19                     # D_in tiles after padding 2312 -> 2432
DINP = KP * 128
MT = DH // 128              # 45 feature tiles
KT2 = MT - 1                # k-tiles covered by DR pairs (44) + 1 single
GRP = [12, 12, 12, 4, 4, 1]  # scan chunk groups (fine tail -> short lag)
G0 = np.cumsum([0] + GRP)
BETA, THRESH = 0.9, 1.0
NUM_CLASSES, TRI_NUM = 10, 45
S_W2L = 2.0**23             # W2 residual prescale

G2_HI = "fp16"              # 'fp16' (e4m3 moving) — f32r rejected w/ e4m3

_nc_cache = {}
_prep_cache = {}


def _register_lif_op():
    """Fused LIF membrane update: out = s0*in0 + in1 - (in0 > 0)."""
    name = "LIF_STEP_ANT"
    for o in dve_ops.OPS:
        if o.name == name:
            return o
    spec = Spec(
        body=(Src0 * C0) + Src1 - (Src0 > Zero),
        reference=lambda in0, in1, s0, s1, imm2: in0.astype(np.float32) * s0
        + in1.reshape(in0.shape)
        - (in0 > 0).astype(np.float32),
    )
    row = max(dve_ops._SUB_OPCODE_FOR_NAME.values()) + 1
    shas = {}
    for ver in ("v3", "v4"):
        uops = dve_lower(spec, ver=ver)
        shas[ver] = DveOpSpec(name=name, opcode=row, uops=uops, rd1_en=True).sha(ver)
    op = dve_ops.DveOp(name, spec, subdim=False, uops_sha=shas)
    dve_ops.OPS.append(op)
    dve_ops.CUSTOM_DVE_SPECS[name] = spec
    dve_ops._SUB_OPCODE_FOR_NAME[name] = row
    return op


LIF_OP = _register_lif_op()


def _build(T):
    """Build + compile the per-core program (same program on all 8 cores)."""
    R = T * BL
    nc = bacc.Bacc(None, target_bir_lowering=False)

    XCH = [(0, 2), (2, 7), (7, 12), (12, 19)]
    xhi_ds = [nc.dram_tensor(f"xhi{i}", [128, c1 - c0, R], FP16,
                             kind="ExternalInput")
              for i, (c0, c1) in enumerate(XCH)]
    xlo_ds = [nc.dram_tensor(f"xlo{i}", [128, c1 - c0, R], FP16,
                             kind="ExternalInput")
              for i, (c0, c1) in enumerate(XCH)]
    whi_d = nc.dram_tensor("whi", [MT, 128, KP, 128], FP16, kind="ExternalInput")
    wlo_d = nc.dram_tensor("wlo", [MT, 128, KP, 128], FP16, kind="ExternalInput")
    w2h_d = nc.dram_tensor("w2h", [MT, 128, MT, 128], FP16, kind="ExternalInput")
    w2l_d = nc.dram_tensor("w2l", [MT, 128, MT, 128], E4M3,
                           kind="ExternalInput")
    w3_d = nc.dram_tensor("w3", [128, MT, DOUT], FP16, kind="ExternalInput")
    b12_d = nc.dram_tensor("b12", [128, 2 * MT], F32, kind="ExternalInput")
    h3o = nc.dram_tensor("h3", [DOUT, R], F32, kind="ExternalOutput")

    add, mult = mybir.AluOpType.add, mybir.AluOpType.mult

    with tile.TileContext(nc) as tc:
        with (
            tc.tile_pool(name="acts", bufs=1) as acts,
            tc.tile_pool(name="wpool", bufs=4) as wpool,
            tc.tile_pool(name="small", bufs=1) as small,
            tc.tile_pool(name="psum", bufs=2, space="PSUM") as pp,
        ):
            xhi = acts.tile([128, KP, R], FP16, tag="xhi")
            xlo = acts.tile([128, KP, R], FP16, tag="xlo")
            tmp1 = acts.tile([128, R], F32, tag="tmp1")
            hsb = acts.tile([128, MT, R], F32, tag="h")
            b12sb = small.tile([128, 2 * MT], F32)
            sts = [small.tile([128, GRP[g], 8 * BL], F32, name=f"st{g}")
                   for g in range(len(GRP))]

            nc.sync.dma_start(out=b12sb[:], in_=b12_d.ap())
            # x pieces: contiguous per-chunk tensors on the gpsimd queue
            for i, (c0, c1) in enumerate(XCH):
                nc.gpsimd.dma_start(out=xhi[:, c0:c1, :], in_=xhi_ds[i].ap())
            for i, (c0, c1) in enumerate(XCH):
                nc.gpsimd.dma_start(out=xlo[:, c0:c1, :], in_=xlo_ds[i].ap())

            # ---- GEMM1 fused: psB = xhi @ Wlo16 (x 2^10 scale);
            #      psA = (xhi + xlo) @ Whi16;  h1 = psA + psB*2^-10 + b1 ----
            with nc.named_scope("gemm1"):
                for m in range(MT):
                    wlo = wpool.tile([128, KP, 128], FP16, tag="w")
                    whi = wpool.tile([128, KP, 128], FP16, tag="w")
                    nc.sync.dma_start(out=wlo[:], in_=wlo_d.ap()[m])
                    nc.sync.dma_start(out=whi[:], in_=whi_d.ap()[m])
                    psB = pp.tile([128, R], F32, tag="psB")
                    for k in range(KP):
                        nc.tensor.matmul(
                            psB[:], wlo[:, k, :], xhi[:, k, :],
                            start=(k == 0), stop=(k == KP - 1),
                        )
                    psA = pp.tile([128, R], F32, tag="psA")
                    for k in range(KP):
                        nc.tensor.matmul(
                            psA[:], whi[:, k, :], xhi[:, k, :],
                            start=(k == 0), stop=False,
                        )
                    for k in range(KP):
                        nc.tensor.matmul(
                            psA[:], whi[:, k, :], xlo[:, k, :],
                            start=False, stop=(k == KP - 1),
                        )
                    # tmp = psB*2^-10 + b1col on Scalar; hsb = tmp + psA on DVE
                    nc.scalar.activation(
                        tmp1[:], psB[:], Ident,
                        bias=b12sb[:, m : m + 1], scale=1.0 / 1024.0,
                    )
                    nc.vector.tensor_tensor(
                        hsb[:, m, :], tmp1[:], psA[:], op=add
                    )

            # spikes, sign-encoded (+1/-1), e4m3 (exact in fp8)
            g1sb = acts.tile([128, MT, R], E4M3, tag="g1")
            g2sb = acts.tile([128, MT, R], E4M3, tag="xhi")  # alias: xhi dead

            def lif_scan(scope, gsb):
                # group g depends only on its own feature tiles, so it
                # starts as soon as the producing GEMM has evicted tiles
                # [G0[g], G0[g+1]) and hides under the GEMM tail.
                with nc.named_scope(scope):
                    for g, gw in enumerate(GRP):
                        st = sts[g]
                        nc.vector.memset(st[:, :, 7 * BL : 8 * BL], -1.0)
                        for t in range(T):
                            hsl = hsb[:, G0[g] : G0[g + 1],
                                      BL * t : BL * (t + 1)]
                            so, do = ((t - 1) % 8) * BL, (t % 8) * BL
                            nc.vector._custom_dve(
                                LIF_OP,
                                out=st[:, :, do : do + BL],
                                in0=st[:, :, so : so + BL],
                                in1=hsl, s0=BETA,
                            )
                            if t % 4 == 3:
                                bo = ((t - 3) % 8) * BL
                                nc.scalar.sign(
                                    gsb[:, G0[g] : G0[g + 1],
                                        BL * (t - 3) : BL * (t + 1)],
                                    st[:, :, bo : bo + 4 * BL],
                                )

            lif_scan("scan1", g1sb)

            # ---- GEMM2: chainA = g1 @ W2hi (fp16); chainB = DR fp8 lo ----
            with nc.named_scope("gemm2"):
                for m in range(MT):
                    w2ht = wpool.tile([128, MT, 128], FP16, tag="w")
                    w2lt = wpool.tile([128, MT, 128], E4M3, tag="w")
                    nc.sync.dma_start(out=w2ht[:], in_=w2h_d.ap()[m])
                    nc.sync.dma_start(out=w2lt[:], in_=w2l_d.ap()[m])
                    psA = pp.tile([128, R], F32, tag="psA")
                    for k in range(MT):
                        nc.tensor.matmul(
                            psA[:], w2ht[:, k, :], g1sb[:, k, :],
                            start=(k == 0), stop=(k == MT - 1),
                        )
                    psB = pp.tile([128, R], F32, tag="psB")
                    for j in range(KT2 // 2):
                        nc.tensor.matmul(
                            psB[:], w2lt[:, 2 * j : 2 * j + 2, :],
                            g1sb[:, 2 * j : 2 * j + 2, :],
                            start=(j == 0), stop=False,
                            perf_mode=DR,
                        )
                    nc.tensor.matmul(
                        psB[:], w2lt[:, MT - 1, :], g1sb[:, MT - 1, :],
                        start=False, stop=True,
                    )
                    nc.vector.tensor_scalar(
                        hsb[:, m, :], psB[:], 1.0 / S_W2L,
                        b12sb[:, MT + m : MT + m + 1], mult, add,
                    )
                    nc.vector.tensor_tensor(
                        hsb[:, m, :], hsb[:, m, :], psA[:], op=add
                    )

            lif_scan("scan2", g2sb)

            # ---- GEMM3: h3 = g2 @ W3h (mixed, single pass), out [90, R] ----
            with nc.named_scope("gemm3"):
                w3sb = wpool.tile([128, MT, DOUT], FP16, tag="w")
                nc.sync.dma_start(out=w3sb[:], in_=w3_d.ap())
                ps3 = pp.tile([DOUT, R], F32, tag="ps3")
                for k in range(MT):
                    nc.tensor.matmul(
                        ps3[:], w3sb[:, k, :], g2sb[:, k, :],
                        start=(k == 0), stop=(k == MT - 1),
                    )
                h3sb = small.tile([DOUT, R], F32, tag="h3sb")
                nc.vector.tensor_copy(h3sb[:], ps3[:])
                nc.sync.dma_start(out=h3o.ap(), in_=h3sb[:])

    nc.compile()
    return nc


def _f16(a):
    return np.asarray(a, np.float32).astype(np.float16)


def _e4m3(a):
    return np.asarray(a, np.float32).astype(ml_dtypes.float8_e4m3)


def _round12(a):
    """Round fp32 to 12-bit significand, RNE — the f32r PE operand grid."""
    u = np.ascontiguousarray(a, np.float32).view(np.uint32)
    u = (u + 0x7FF + ((u >> 12) & 1)) & np.uint32(0xFFFFF000)
    return u.view(np.float32)


def _prep_weights(fc1_w, fc1_b, fc2_w, fc2_b, fco_w):
    key = (fc1_w.ctypes.data, fc2_w.ctypes.data, fco_w.ctypes.data)
    if key in _prep_cache:
        return _prep_cache[key]
    # GEMM1: W1 = fp16 hi + fp16(residual * 2^10) (prescale dodges fp16
    # subnormal flushing; 22-bit total, products exact, ~1.6e-7 rms)
    W1p = np.zeros((DH, DINP), np.float32)
    W1p[:, :DIN] = fc1_w
    W1t = np.ascontiguousarray(
        W1p.reshape(MT, 128, KP, 128).transpose(0, 3, 2, 1)
    ).astype(np.float64)  # [m, p, k, q] = W1[m*128+q, k*128+p]
    whi = _f16(W1t)
    wlo = np.ascontiguousarray(_f16((W1t - whi.astype(np.float64)) * 1024.0))
    # GEMM2: W2/2 = fp16 hi + e4m3(residual * 2^23), sign-encoded spikes
    W2t = (fc2_w.reshape(MT, 128, MT, 128).transpose(0, 3, 2, 1)
           .astype(np.float64) * 0.5)
    w2h = np.ascontiguousarray(_f16(W2t))
    w2l45 = _e4m3((W2t - w2h.astype(np.float64)) * S_W2L)  # [m, p, 45, q]
    w2l = np.ascontiguousarray(w2l45)
    # GEMM3: W3/2 single fp16
    W3t = (fco_w.reshape(DOUT, MT, 128).transpose(2, 1, 0)
           .astype(np.float64) * 0.5)  # [p, k, q]
    w3 = np.ascontiguousarray(_f16(W3t))
    # biases: threshold shift -(1-beta); sign-encoding rowsum corrections
    # from the actually-shipped weight values (fp64 for exactness)
    c2 = (w2h.astype(np.float64)
          + w2l45.astype(np.float64) / S_W2L).sum(axis=(1, 2))  # [m, q]
    b2c = (fc2_b.astype(np.float64).reshape(MT, 128)
           - (1.0 - BETA) * THRESH + c2).astype(np.float32)
    b1s = (fc1_b - (1.0 - BETA) * THRESH).reshape(MT, 128).T
    b12 = np.ascontiguousarray(
        np.concatenate([b1s, b2c.T], axis=1).astype(np.float32))
    b3c = w3.astype(np.float64).sum(axis=(0, 1))  # [DOUT]
    out = dict(
        inputs=dict(whi=whi, wlo=wlo, w2h=w2h, w2l=w2l, w3=w3, b12=b12),
        b3c=b3c.astype(np.float32),
    )
    _prep_cache[key] = out
    return out


def _prep_x(x, T):
    """Per-core x arrays [128, KP, R] (r = t*BL + b), 12-bit hi/lo split."""
    xf = np.asarray(x, np.float32).reshape(B, T, -1)
    outs = []
    for c in range(NCORES):
        xc = xf[BL * c : BL * (c + 1)]            # [BL, T, DIN]
        xp = np.zeros((DINP, T * BL), np.float32)
        xp[:DIN] = xc.transpose(2, 1, 0).reshape(DIN, T * BL)
        xt = np.ascontiguousarray(
            xp.reshape(KP, 128, T * BL).transpose(1, 0, 2))
        xhi = _f16(xt)
        xlo = _f16(xt.astype(np.float64) - xhi.astype(np.float64))
        XCH = [(0, 2), (2, 7), (7, 12), (12, 19)]
        d = {}
        for i, (c0, c1) in enumerate(XCH):
            d[f"xhi{i}"] = np.ascontiguousarray(xhi[:, c0:c1, :])
            d[f"xlo{i}"] = np.ascontiguousarray(xlo[:, c0:c1, :])
        outs.append(d)
    return outs


def kernel(x, fc1_w, fc1_b, fc2_w, fc2_b, fco_w, fco_b, _T=None,
           _want_results=False, _trace=False):
    T = _T or T_FULL
    if T not in _nc_cache:
        _nc_cache[T] = _build(T)
    nc = _nc_cache[T]

    w = _prep_weights(
        np.asarray(fc1_w, np.float32), np.asarray(fc1_b, np.float32),
        np.asarray(fc2_w, np.float32), np.asarray(fc2_b, np.float32),
        np.asarray(fco_w, np.float32),
    )
    xs = _prep_x(x, T)
    in_maps = [{**xs[c], **w["inputs"]} for c in range(NCORES)]
    res = run_bass_kernel_spmd(nc, in_maps, list(range(NCORES)), trace=_trace)

    # host: output-layer LIF scan + T-sum + pairwise voting (exact fp32)
    h3 = np.stack([res.results[c]["h3"] for c in range(NCORES)])  # [8, 90, R]
    i3 = h3.reshape(NCORES, DOUT, T, BL) \
        + (np.asarray(fco_b, np.float32) + w["b3c"])[None, :, None, None]
    i3 = i3.transpose(2, 0, 3, 1).reshape(T, B, DOUT)  # [T, 32, 90]
    m = np.zeros((B, DOUT), np.float32)
    s = np.zeros((B, DOUT), np.float32)
    out = np.zeros((B, DOUT), np.float32)
    for t in range(T):
        m = BETA * m + i3[t] - s * THRESH
        s = ((m - THRESH) > 0).astype(np.float32)
        out += s
    pi, pj = np.triu_indices(NUM_CLASSES, 1)
    outp = out.reshape(B, TRI_NUM, 2)
    votes = np.zeros((B, NUM_CLASSES), np.float32)
    np.add.at(votes, (slice(None), pi), outp[..., 0])
    np.add.at(votes, (slice(None), pj), outp[..., 1])
    if _want_results:
        return votes, res
    return votes
